# revision 2
# baseline (speedup 1.0000x reference)
"""Chamfer bidirectional nearest-neighbor (dist + argmin idx) for
B=8, N=M=8192, D=3 on 8 Trainium2 NeuronCores, data-parallel over batch
(core b handles batch b; no cross-core communication needed).

Math per core, reference formula: d[n,m] = (sq1[n]+sq2[m]) - 2*cross[n,m].

Two kernel variants:
  'e4'  : PE computes e[n,m] = sq2[m] - 2*cross[n,m] directly via a K=4
          matmul (rows: x_q coords with moving -2*x_db coords; ones row with
          moving sq_db row). argmin_m d == argmin_m e since sq1[n] is a
          per-row constant. VectorE does tensor_reduce(min) + max_index
          straight from PSUM (2 streams/element), dist = sq1[n] + e_min.
  'stt' : PE computes cross2 = 2*cross (K=3); VectorE scalar_tensor_tensor
          computes d = (sq_db_bcast + sq_q) - cross2 with the reference's
          exact fp32 association, then reduce(min) + max_index (3 streams).

argmin ties resolve to the first occurrence (max_index is a first-match
value scan), matching jnp.argmin.
"""
import numpy as np

B, N, M, D = 8, 8192, 8192, 3
P = 128
CH = 512          # one PSUM bank of fp32
SC = 2048         # super-chunk: 4 banks
NT = N // P       # 64 query tiles
NSC = M // SC     # 4 super-chunks per row
NC_CORES = 8
VARIANT = 'stt'   # 'e4' (fast approx) or 'stt' (bit-exact vs reference)
WORK_BUFS = 3     # dtile buffering depth (ACT write / DMA RMW / DVE reduce+scan)
PSUM_BUFS = 3     # PSUM pool depth (PSUM_BUFS * STT_BANKS banks)
STT_BANKS = 2     # PSUM banks consumed per STT instruction (width 512*STT_BANKS)
USE_DMA_ACCUM = False  # d-compute on ScalarE + accumulate-DMA (DVE: 2 streams)

_CACHE = {}


def _legalize_waits(nc):
    """This walrus build encodes ONE wait slot per TPB instruction
    (NEURON_ISA_TPB_EVENTS); hoist excess semaphore waits onto injected
    same-engine NoOps placed just before the instruction. Drain has no
    wait slot at all. DMA completion updates are never moved."""
    import concourse.mybir as mybir

    counter = [0]

    def mknop(engine, wait):
        counter[0] += 1
        nop = mybir.InstNoOp(name=f'I-lgw-{counter[0]}', ins=[], outs=[])
        nop.engine = engine
        nop.sync_info = mybir.SyncInfo(on_wait=[wait], on_update=[])
        return nop

    for f in nc.m.functions:
        for b in f.blocks:
            new_insts = []
            for ins in b.instructions:
                si = ins.sync_info
                waits = list(si.on_wait) if si is not None and si.on_wait else []
                limit = 0 if ins.opcode == 'Drain' else 1
                if len(waits) > limit:
                    keep, hoist = [], []
                    for w in waits:
                        if len(keep) < limit and getattr(w, 'wait_reg', None) is not None:
                            keep.append(w)
                        else:
                            hoist.append(w)
                    while len(keep) < limit and hoist:
                        keep.append(hoist.pop(0))
                    for w in hoist:
                        new_insts.append(mknop(ins.engine, w))
                    ins.sync_info = mybir.SyncInfo(
                        on_wait=keep,
                        on_update=list(si.on_update) if si.on_update else [])
                new_insts.append(ins)
            b.instructions = new_insts


def _emit_direction_e4(nc, pool, work, pp, lhs_dram, rhs_dram, sqq_dram,
                       iota8_dram, dist_dram, idx_dram, tag):
    import concourse.mybir as mybir
    F32 = mybir.dt.float32
    U32 = mybir.dt.uint32
    AX = mybir.AxisListType
    OP = mybir.AluOpType

    lhs = pool.tile([D + 1, N], F32, tag=f'lhs{tag}')
    nc.sync.dma_start(out=lhs[:], in_=lhs_dram[:])
    rhs = pool.tile([D + 1, M], F32, tag=f'rhs{tag}')
    nc.sync.dma_start(out=rhs[:], in_=rhs_dram[:])
    sqq = pool.tile([P, NT], F32, tag=f'sqq{tag}')
    nc.sync.dma_start(out=sqq[:], in_=sqq_dram[:].rearrange('(t p) -> p t', p=P))
    iota8 = pool.tile([P, 8], F32, tag=f'iota8{tag}')
    nc.sync.dma_start(out=iota8[:], in_=iota8_dram[:].unsqueeze(0).to_broadcast((P, 8)))

    dist_acc = pool.tile([P, NT], F32, tag=f'dacc{tag}')
    idx_acc = pool.tile([P, NT], U32, tag=f'iacc{tag}')

    for t in range(NT):
        scv = work.tile([P, 8], F32, tag='scv')       # super-chunk mins (cols 4..7 = +inf)
        sci = work.tile([P, 8], F32, tag='sci')       # super-chunk argmins as f32
        nc.vector.memset(scv[:, NSC:8], 3.0e38)
        for s in range(NSC):
            ep = pp.tile([P, SC], F32, tag='ep')
            for c in range(SC // CH):
                off = s * SC + c * CH
                nc.tensor.matmul(ep[:, c * CH:(c + 1) * CH],
                                 lhsT=lhs[:, t * P:(t + 1) * P],
                                 rhs=rhs[:, off:off + CH], start=True, stop=True)
            nc.vector.tensor_reduce(scv[:, s:s + 1], ep[:], axis=AX.X, op=OP.min)
            m8 = work.tile([P, 8], F32, tag='m8')
            nc.vector.tensor_copy(m8[:], scv[:, s:s + 1].to_broadcast((P, 8)))
            i8 = work.tile([P, 8], U32, tag='i8')
            nc.vector.max_index(out=i8[:], in_max=m8[:], in_values=ep[:])
            nc.vector.tensor_copy(sci[:, s:s + 1], i8[:, 0:1])   # u32 -> f32 cast
        # combine: global min, first super-chunk achieving it, its local idx
        rowmin = work.tile([P, 1], F32, tag='rowmin')
        nc.vector.tensor_reduce(rowmin[:], scv[:, 0:NSC], axis=AX.X, op=OP.min)
        rm8 = work.tile([P, 8], F32, tag='rm8')
        nc.vector.tensor_copy(rm8[:], rowmin[:].to_broadcast((P, 8)))
        s8 = work.tile([P, 8], U32, tag='s8')
        nc.vector.max_index(out=s8[:], in_max=rm8[:], in_values=scv[:])
        sf = work.tile([P, 1], F32, tag='sf')
        nc.vector.tensor_copy(sf[:], s8[:, 0:1])                 # u32 -> f32 cast
        oh = work.tile([P, 8], F32, tag='oh')
        nc.vector.tensor_scalar(out=oh[:], in0=iota8[:], scalar1=sf[:], scalar2=None,
                                op0=OP.is_equal)
        ohsci = work.tile([P, 8], F32, tag='ohsci')
        nc.vector.tensor_mul(ohsci[:], oh[:], sci[:])
        idxf = work.tile([P, 1], F32, tag='idxf')
        nc.vector.tensor_reduce(idxf[:], ohsci[:], axis=AX.X, op=OP.add)
        # idx = sci[s*] + SC * s*
        sbase = work.tile([P, 1], F32, tag='sbase')
        nc.vector.tensor_scalar(out=sbase[:], in0=sf[:], scalar1=float(SC), scalar2=None,
                                op0=OP.mult)
        nc.vector.tensor_add(idxf[:], idxf[:], sbase[:])
        nc.vector.tensor_copy(idx_acc[:, t:t + 1], idxf[:])      # f32 -> u32 cast
        # dist = sq_q[n] + rowmin
        nc.vector.tensor_scalar(out=dist_acc[:, t:t + 1], in0=rowmin[:],
                                scalar1=sqq[:, t:t + 1], scalar2=None, op0=OP.add)

    nc.sync.dma_start(out=dist_dram[:].rearrange('(t p) -> p t', p=P), in_=dist_acc[:])
    nc.sync.dma_start(out=idx_dram[:].rearrange('(t p) -> p t', p=P), in_=idx_acc[:])


def _emit_direction_stt(nc, pool, work, pp, lhs_dram, rhs_dram, sqq_dram,
                        sqdb_dram, dist_dram, idx_dram, tag, nt_loop=None):
    """Exact-association variant: d = (sq_db_bcast + sq_q) - cross2."""
    import concourse.mybir as mybir
    F32 = mybir.dt.float32
    U32 = mybir.dt.uint32
    AX = mybir.AxisListType
    OP = mybir.AluOpType

    lhs = pool.tile([D, N], F32, tag='lhsS')
    nc.sync.dma_start(out=lhs[:], in_=lhs_dram[0:D, :])
    rhs = pool.tile([D, M], F32, tag='rhsS')
    nc.sync.dma_start(out=rhs[:], in_=rhs_dram[0:D, :])
    sqq = pool.tile([P, NT], F32, tag=f'sqq{tag}')
    nc.sync.dma_start(out=sqq[:], in_=sqq_dram[:].rearrange('(t p) -> p t', p=P))
    sqdb_bc = pool.tile([P, M], F32, tag='sqdbS')
    nc.sync.dma_start(out=sqdb_bc[:], in_=sqdb_dram[:].unsqueeze(0).to_broadcast((P, M)))

    dist_acc = pool.tile([P, NT], F32, tag=f'dacc{tag}')
    idx_acc = pool.tile([P, NT], U32, tag=f'iacc{tag}')

    AF = mybir.ActivationFunctionType
    CW = STT_BANKS * CH  # STT width: STT_BANKS PSUM banks per instruction
    NCHUNK = M // CW
    for t in range(NT if nt_loop is None else nt_loop):
        dtile = work.tile([P, M], F32, tag='dtile')
        for c in range(NCHUNK):
            ps = pp.tile([P, CW], F32, tag='ps')
            for h in range(STT_BANKS):
                nc.tensor.matmul(ps[:, h * CH:(h + 1) * CH],
                                 lhsT=lhs[:, t * P:(t + 1) * P],
                                 rhs=rhs[:, c * CW + h * CH:c * CW + (h + 1) * CH],
                                 start=True, stop=True)
            # ps holds -2*cross (rhs rows are -2*x_db), so ADD it:
            # d = (sq_db + sq_q) + (-2cross)  ==  fl(sq12 - 2cross) bitwise
            if USE_DMA_ACCUM:
                # ScalarE: dtile <- sq12 (Identity+bias, exact); cross copied
                # out of PSUM; SWDGE accumulate-DMA adds it. Frees VectorE
                # from the d-compute stream entirely.
                nc.scalar.activation(out=dtile[:, c * CW:(c + 1) * CW],
                                     in_=sqdb_bc[:, c * CW:(c + 1) * CW],
                                     func=AF.Identity, bias=sqq[:, t:t + 1], scale=1.0)
                cs = work.tile([P, CW], F32, tag='cs')
                nc.scalar.activation(out=cs[:], in_=ps[:], func=AF.Copy)
                nc.gpsimd.dma_start(out=dtile[:, c * CW:(c + 1) * CW], in_=cs[:],
                                    accum_op=OP.add)
            else:
                nc.vector.scalar_tensor_tensor(
                    out=dtile[:, c * CW:(c + 1) * CW],
                    in0=sqdb_bc[:, c * CW:(c + 1) * CW],
                    scalar=sqq[:, t:t + 1], in1=ps[:],
                    op0=OP.add, op1=OP.add)
        nc.vector.tensor_reduce(dist_acc[:, t:t + 1], dtile[:], axis=AX.X, op=OP.min)
        rm8 = work.tile([P, 8], F32, tag='rm8')
        nc.vector.tensor_copy(rm8[:], dist_acc[:, t:t + 1].to_broadcast((P, 8)))
        i8 = work.tile([P, 8], U32, tag='i8')
        nc.vector.max_index(out=i8[:], in_max=rm8[:], in_values=dtile[:])
        nc.vector.tensor_copy(idx_acc[:, t:t + 1], i8[:, 0:1])

    ntl = NT if nt_loop is None else nt_loop
    nc.sync.dma_start(out=dist_dram[0:ntl * P].rearrange('(t p) -> p t', p=P),
                      in_=dist_acc[:, 0:ntl])
    nc.sync.dma_start(out=idx_dram[0:ntl * P].rearrange('(t p) -> p t', p=P),
                      in_=idx_acc[:, 0:ntl])


def _build(variant):
    import concourse.bass as bass
    import concourse.mybir as mybir
    from concourse.tile import TileContext
    F32 = mybir.dt.float32
    U32 = mybir.dt.uint32

    nc = bass.Bass()
    lhs1 = nc.dram_tensor('lhs1', [D + 1, N], F32, kind='ExternalInput')
    rhs1 = nc.dram_tensor('rhs1', [D + 1, M], F32, kind='ExternalInput')
    lhs2 = nc.dram_tensor('lhs2', [D + 1, M], F32, kind='ExternalInput')
    rhs2 = nc.dram_tensor('rhs2', [D + 1, N], F32, kind='ExternalInput')
    sq1_d = nc.dram_tensor('sq1', [N], F32, kind='ExternalInput')
    sq2_d = nc.dram_tensor('sq2', [M], F32, kind='ExternalInput')
    iota8_d = nc.dram_tensor('iota8', [8], F32, kind='ExternalInput')
    dist1 = nc.dram_tensor('dist1', [N], F32, kind='ExternalOutput')
    dist2 = nc.dram_tensor('dist2', [M], F32, kind='ExternalOutput')
    idx1 = nc.dram_tensor('idx1', [N], U32, kind='ExternalOutput')
    idx2 = nc.dram_tensor('idx2', [M], U32, kind='ExternalOutput')

    with TileContext(nc) as tc:
        with tc.tile_pool(name='pool', bufs=1) as pool, \
             tc.tile_pool(name='work', bufs=WORK_BUFS) as work, \
             tc.tile_pool(name='psum', bufs=PSUM_BUFS, space='PSUM') as pp:
            if variant == 'e4':
                _emit_direction_e4(nc, pool, work, pp, lhs1, rhs1, sq1_d,
                                   iota8_d, dist1, idx1, tag='1')
                _emit_direction_e4(nc, pool, work, pp, lhs2, rhs2, sq2_d,
                                   iota8_d, dist2, idx2, tag='2')
            else:
                _emit_direction_stt(nc, pool, work, pp, lhs1, rhs1, sq1_d,
                                    sq2_d, dist1, idx1, tag='1')
                _emit_direction_stt(nc, pool, work, pp, lhs2, rhs2, sq2_d,
                                    sq1_d, dist2, idx2, tag='2')
    _legalize_waits(nc)
    return nc


def _sq_rows(a):
    # fp32 sequential sum of squares along last axis; bit-matches the
    # device reference's multiply+reduce_sum
    return ((a[:, 0] * a[:, 0] + a[:, 1] * a[:, 1]).astype(np.float32)
            + a[:, 2] * a[:, 2]).astype(np.float32)


def _host_pack(x_q, x_db, sq_db):
    lhs = np.empty((D + 1, x_q.shape[0]), np.float32)
    lhs[0:D] = x_q.T
    lhs[D] = 1.0
    rhs = np.empty((D + 1, x_db.shape[0]), np.float32)
    rhs[0:D] = -2.0 * x_db.T
    rhs[D] = sq_db
    return np.ascontiguousarray(lhs), np.ascontiguousarray(rhs)


def _make_in_maps(xyz1, xyz2):
    iota8 = np.arange(8, dtype=np.float32)
    in_maps = []
    for b in range(NC_CORES):
        x1 = np.ascontiguousarray(xyz1[b])
        x2 = np.ascontiguousarray(xyz2[b])
        sq1 = _sq_rows(x1)
        sq2 = _sq_rows(x2)
        l1, r1 = _host_pack(x1, x2, sq2)
        l2, r2 = _host_pack(x2, x1, sq1)
        in_maps.append({'lhs1': l1, 'rhs1': r1, 'lhs2': l2, 'rhs2': r2,
                        'sq1': sq1, 'sq2': sq2, 'iota8': iota8})
    return in_maps


def _make_runner(nc):
    """Build the jitted shard_map callable ONCE (mirrors
    bass2jax.run_bass_via_pjrt's multi-core branch); repeated calls then
    skip tracing/BIR-serialization/compile and only pay transfer+execute."""
    import jax
    import numpy as _np
    from jax.experimental.shard_map import shard_map
    from jax.sharding import Mesh, PartitionSpec
    import concourse.mybir as mybir
    from concourse import bass2jax

    bass2jax.install_neuronx_cc_hook()

    partition_name = nc.partition_id_tensor.name if nc.partition_id_tensor else None
    in_names, out_names, out_avals, zero_outs = [], [], [], []
    for alloc in nc.m.functions[0].allocations:
        if not isinstance(alloc, mybir.MemoryLocationSet):
            continue
        name = alloc.memorylocations[0].name
        if alloc.kind == 'ExternalInput':
            if name != partition_name:
                in_names.append(name)
        elif alloc.kind == 'ExternalOutput':
            assert alloc.tensor_shape is not None and alloc.dtype is not None
            out_names.append(name)
            out_avals.append(jax.core.ShapedArray(
                tuple(alloc.tensor_shape), mybir.dt.np(alloc.dtype)))
            zero_outs.append(_np.zeros(tuple(alloc.tensor_shape),
                                       mybir.dt.np(alloc.dtype)))

    n_params = len(in_names)
    n_outs = len(out_names)
    all_names = list(in_names) + list(out_names)
    if partition_name is not None:
        all_names.append(partition_name)
    donate = tuple(range(n_params, n_params + n_outs))

    def _body(*args):
        operands = list(args)
        if partition_name is not None:
            operands.append(bass2jax.partition_id_tensor())
        outs = bass2jax._bass_exec_p.bind(
            *operands,
            out_avals=tuple(out_avals),
            in_names=tuple(all_names),
            out_names=tuple(out_names),
            lowering_input_output_aliases=(),
            sim_require_finite=True,
            sim_require_nnan=True,
            nc=nc,
        )
        return tuple(outs)

    devices = jax.devices()[:NC_CORES]
    mesh = Mesh(np.asarray(devices), ('core',))
    in_specs = (PartitionSpec('core'),) * (n_params + n_outs)
    out_specs = (PartitionSpec('core'),) * n_outs
    sharded = jax.jit(
        shard_map(_body, mesh=mesh, in_specs=in_specs, out_specs=out_specs,
                  check_rep=False),
        donate_argnums=donate, keep_unused=True)

    def runner(in_maps):
        concat_in = [
            np.concatenate([np.asarray(in_maps[c][nm]) for c in range(NC_CORES)],
                           axis=0)
            for nm in in_names]
        concat_zeros = [np.zeros((NC_CORES * z.shape[0], *z.shape[1:]), z.dtype)
                        for z in zero_outs]
        out_arrs = sharded(*concat_in, *concat_zeros)
        return [
            {nm: np.asarray(out_arrs[i]).reshape(NC_CORES, *out_avals[i].shape)[c]
             for i, nm in enumerate(out_names)}
            for c in range(NC_CORES)]

    return runner


class _Res:
    def __init__(self, results):
        self.results = results
        self.exec_time_ns = None


def _run(in_maps, **kwargs):
    key = f'nc_{VARIANT}'
    if key not in _CACHE:
        _CACHE[key] = _build(VARIANT)
    rkey = f'runner_{VARIANT}'
    if rkey not in _CACHE:
        _CACHE[rkey] = _make_runner(_CACHE[key])
    return _Res(_CACHE[rkey](in_maps))


def kernel(xyz1: np.ndarray, xyz2: np.ndarray):
    xyz1 = np.asarray(xyz1, dtype=np.float32)
    xyz2 = np.asarray(xyz2, dtype=np.float32)
    res = _run(_make_in_maps(xyz1, xyz2))
    outs = res.results
    dist1 = np.stack([outs[b]['dist1'] for b in range(B)])
    dist2 = np.stack([outs[b]['dist2'] for b in range(B)])
    idx1 = np.stack([outs[b]['idx1'] for b in range(B)]).view(np.int32)
    idx2 = np.stack([outs[b]['idx2'] for b in range(B)]).view(np.int32)
    return dist1, dist2, idx1, idx2


def _build_tiny():
    """Minimal kernel through the same path, for overhead calibration."""
    import concourse.bass as bass
    import concourse.mybir as mybir
    from concourse.tile import TileContext
    F32 = mybir.dt.float32
    nc = bass.Bass()
    a = nc.dram_tensor('lhs1', [D + 1, N], F32, kind='ExternalInput')
    o = nc.dram_tensor('tinyout', [D + 1, 128], F32, kind='ExternalOutput')
    with TileContext(nc) as tc:
        with tc.tile_pool(name='pool', bufs=1) as pool:
            t = pool.tile([D + 1, 128], F32)
            nc.sync.dma_start(out=t[:], in_=a[:, 0:128])
            nc.sync.dma_start(out=o[:], in_=t[:])
    _legalize_waits(nc)
    return nc


def timed_run(np_inputs, iters=10):
    """Estimate on-device exec time: warm wall-clock of the full kernel minus
    warm wall-clock of a tiny kernel through the identical cached-jit path.
    (No NTFF profiling hook is available under this axon client.)"""
    import time
    in_maps = _make_in_maps(np_inputs['xyz1'], np_inputs['xyz2'])
    if 'tiny' not in _CACHE:
        _CACHE['tiny'] = _build_tiny()
    if 'tiny_runner' not in _CACHE:
        _CACHE['tiny_runner'] = _make_runner(_CACHE['tiny'])
    tiny_maps = [{'lhs1': m['lhs1']} for m in in_maps]
    # warm both
    _run(in_maps)
    _CACHE['tiny_runner'](tiny_maps)
    full_t, tiny_t = [], []
    for _ in range(iters):
        t0 = time.perf_counter()
        _run(in_maps)
        full_t.append(time.perf_counter() - t0)
        t0 = time.perf_counter()
        _CACHE['tiny_runner'](tiny_maps)
        tiny_t.append(time.perf_counter() - t0)
    full_ns = min(full_t) * 1e9
    tiny_ns = min(tiny_t) * 1e9
    print(f'full wall (warm, cached jit): {full_ns/1e6:.3f} ms, '
          f'tiny wall (dispatch overhead): {tiny_ns/1e6:.3f} ms')
    return int(full_ns - tiny_ns)



# revision 4
# speedup vs baseline: 1.0668x; 1.0668x over previous
"""Chamfer bidirectional nearest-neighbor (dist + argmin idx) for
B=8, N=M=8192, D=3 on 8 Trainium2 NeuronCores, data-parallel over batch
(core b handles batch b; no cross-core communication needed).

Math per core, reference formula: d[n,m] = (sq1[n]+sq2[m]) - 2*cross[n,m].

Two kernel variants:
  'e4'  : PE computes e[n,m] = sq2[m] - 2*cross[n,m] directly via a K=4
          matmul (rows: x_q coords with moving -2*x_db coords; ones row with
          moving sq_db row). argmin_m d == argmin_m e since sq1[n] is a
          per-row constant. VectorE does tensor_reduce(min) + max_index
          straight from PSUM (2 streams/element), dist = sq1[n] + e_min.
  'stt' : PE computes cross2 = 2*cross (K=3); VectorE scalar_tensor_tensor
          computes d = (sq_db_bcast + sq_q) - cross2 with the reference's
          exact fp32 association, then reduce(min) + max_index (3 streams).

argmin ties resolve to the first occurrence (max_index is a first-match
value scan), matching jnp.argmin.
"""
import numpy as np

B, N, M, D = 8, 8192, 8192, 3
P = 128
CH = 512          # one PSUM bank of fp32
SC = 2048         # super-chunk: 4 banks
NT = N // P       # 64 query tiles
NSC = M // SC     # 4 super-chunks per row
NC_CORES = 8
VARIANT = 'stt'   # 'e4' (fast approx) or 'stt' (bit-exact vs reference)
WORK_BUFS = 3     # dtile buffering depth (ACT write / DMA RMW / DVE reduce+scan)
PSUM_BUFS = 3     # PSUM pool depth (PSUM_BUFS * STT_BANKS banks)
STT_BANKS = 2     # PSUM banks consumed per STT instruction (width 512*STT_BANKS)
USE_DMA_ACCUM = False  # d-compute on ScalarE + accumulate-DMA (DVE: 2 streams)

import os
REPS = int(os.environ.get('KREPS', '1'))   # repeat compute inside one NEFF (timing probe)

_CACHE = {}


def _legalize_waits(nc):
    """This walrus build encodes ONE wait slot per TPB instruction
    (NEURON_ISA_TPB_EVENTS); hoist excess semaphore waits onto injected
    same-engine NoOps placed just before the instruction. Drain has no
    wait slot at all. DMA completion updates are never moved."""
    import concourse.mybir as mybir

    counter = [0]

    def mknop(engine, wait):
        counter[0] += 1
        nop = mybir.InstNoOp(name=f'I-lgw-{counter[0]}', ins=[], outs=[])
        nop.engine = engine
        nop.sync_info = mybir.SyncInfo(on_wait=[wait], on_update=[])
        return nop

    for f in nc.m.functions:
        for b in f.blocks:
            new_insts = []
            for ins in b.instructions:
                si = ins.sync_info
                waits = list(si.on_wait) if si is not None and si.on_wait else []
                limit = 0 if ins.opcode == 'Drain' else 1
                if len(waits) > limit:
                    keep, hoist = [], []
                    for w in waits:
                        if len(keep) < limit and getattr(w, 'wait_reg', None) is not None:
                            keep.append(w)
                        else:
                            hoist.append(w)
                    while len(keep) < limit and hoist:
                        keep.append(hoist.pop(0))
                    for w in hoist:
                        new_insts.append(mknop(ins.engine, w))
                    ins.sync_info = mybir.SyncInfo(
                        on_wait=keep,
                        on_update=list(si.on_update) if si.on_update else [])
                new_insts.append(ins)
            b.instructions = new_insts


def _emit_direction_e4(nc, pool, work, pp, lhs_dram, rhs_dram, sqq_dram,
                       iota8_dram, dist_dram, idx_dram, tag):
    import concourse.mybir as mybir
    F32 = mybir.dt.float32
    U32 = mybir.dt.uint32
    AX = mybir.AxisListType
    OP = mybir.AluOpType

    lhs = pool.tile([D + 1, N], F32, tag=f'lhs{tag}')
    nc.sync.dma_start(out=lhs[:], in_=lhs_dram[:])
    rhs = pool.tile([D + 1, M], F32, tag=f'rhs{tag}')
    nc.sync.dma_start(out=rhs[:], in_=rhs_dram[:])
    sqq = pool.tile([P, NT], F32, tag=f'sqq{tag}')
    nc.sync.dma_start(out=sqq[:], in_=sqq_dram[:].rearrange('(t p) -> p t', p=P))
    iota8 = pool.tile([P, 8], F32, tag=f'iota8{tag}')
    nc.sync.dma_start(out=iota8[:], in_=iota8_dram[:].unsqueeze(0).to_broadcast((P, 8)))

    dist_acc = pool.tile([P, NT], F32, tag=f'dacc{tag}')
    idx_acc = pool.tile([P, NT], U32, tag=f'iacc{tag}')

    for t in range(NT):
        scv = work.tile([P, 8], F32, tag='scv')       # super-chunk mins (cols 4..7 = +inf)
        sci = work.tile([P, 8], F32, tag='sci')       # super-chunk argmins as f32
        nc.vector.memset(scv[:, NSC:8], 3.0e38)
        for s in range(NSC):
            ep = pp.tile([P, SC], F32, tag='ep')
            for c in range(SC // CH):
                off = s * SC + c * CH
                nc.tensor.matmul(ep[:, c * CH:(c + 1) * CH],
                                 lhsT=lhs[:, t * P:(t + 1) * P],
                                 rhs=rhs[:, off:off + CH], start=True, stop=True)
            nc.vector.tensor_reduce(scv[:, s:s + 1], ep[:], axis=AX.X, op=OP.min)
            m8 = work.tile([P, 8], F32, tag='m8')
            nc.vector.tensor_copy(m8[:], scv[:, s:s + 1].to_broadcast((P, 8)))
            i8 = work.tile([P, 8], U32, tag='i8')
            nc.vector.max_index(out=i8[:], in_max=m8[:], in_values=ep[:])
            nc.vector.tensor_copy(sci[:, s:s + 1], i8[:, 0:1])   # u32 -> f32 cast
        # combine: global min, first super-chunk achieving it, its local idx
        rowmin = work.tile([P, 1], F32, tag='rowmin')
        nc.vector.tensor_reduce(rowmin[:], scv[:, 0:NSC], axis=AX.X, op=OP.min)
        rm8 = work.tile([P, 8], F32, tag='rm8')
        nc.vector.tensor_copy(rm8[:], rowmin[:].to_broadcast((P, 8)))
        s8 = work.tile([P, 8], U32, tag='s8')
        nc.vector.max_index(out=s8[:], in_max=rm8[:], in_values=scv[:])
        sf = work.tile([P, 1], F32, tag='sf')
        nc.vector.tensor_copy(sf[:], s8[:, 0:1])                 # u32 -> f32 cast
        oh = work.tile([P, 8], F32, tag='oh')
        nc.vector.tensor_scalar(out=oh[:], in0=iota8[:], scalar1=sf[:], scalar2=None,
                                op0=OP.is_equal)
        ohsci = work.tile([P, 8], F32, tag='ohsci')
        nc.vector.tensor_mul(ohsci[:], oh[:], sci[:])
        idxf = work.tile([P, 1], F32, tag='idxf')
        nc.vector.tensor_reduce(idxf[:], ohsci[:], axis=AX.X, op=OP.add)
        # idx = sci[s*] + SC * s*
        sbase = work.tile([P, 1], F32, tag='sbase')
        nc.vector.tensor_scalar(out=sbase[:], in0=sf[:], scalar1=float(SC), scalar2=None,
                                op0=OP.mult)
        nc.vector.tensor_add(idxf[:], idxf[:], sbase[:])
        nc.vector.tensor_copy(idx_acc[:, t:t + 1], idxf[:])      # f32 -> u32 cast
        # dist = sq_q[n] + rowmin
        nc.vector.tensor_scalar(out=dist_acc[:, t:t + 1], in0=rowmin[:],
                                scalar1=sqq[:, t:t + 1], scalar2=None, op0=OP.add)

    nc.sync.dma_start(out=dist_dram[:].rearrange('(t p) -> p t', p=P), in_=dist_acc[:])
    nc.sync.dma_start(out=idx_dram[:].rearrange('(t p) -> p t', p=P), in_=idx_acc[:])


def _emit_direction_stt(nc, pool, work, pp, lhs_dram, rhs_dram, sqq_dram,
                        sqdb_dram, dist_dram, idx_dram, tag, nt_loop=None):
    """Exact-association variant: d = (sq_db_bcast + sq_q) - cross2."""
    import concourse.mybir as mybir
    F32 = mybir.dt.float32
    U32 = mybir.dt.uint32
    AX = mybir.AxisListType
    OP = mybir.AluOpType

    lhs = pool.tile([D, N], F32, tag='lhsS')
    nc.sync.dma_start(out=lhs[:], in_=lhs_dram[0:D, :])
    rhs = pool.tile([D, M], F32, tag='rhsS')
    nc.sync.dma_start(out=rhs[:], in_=rhs_dram[0:D, :])
    sqq = pool.tile([P, NT], F32, tag=f'sqq{tag}')
    nc.sync.dma_start(out=sqq[:], in_=sqq_dram[:].rearrange('(t p) -> p t', p=P))
    sqdb_bc = pool.tile([P, M], F32, tag='sqdbS')
    nc.sync.dma_start(out=sqdb_bc[:], in_=sqdb_dram[:].unsqueeze(0).to_broadcast((P, M)))

    dist_acc = pool.tile([P, NT], F32, tag=f'dacc{tag}')
    idx_acc = pool.tile([P, NT], U32, tag=f'iacc{tag}')

    AF = mybir.ActivationFunctionType
    CW = STT_BANKS * CH  # STT width: STT_BANKS PSUM banks per instruction
    NCHUNK = M // CW
    for t in range(NT if nt_loop is None else nt_loop):
        dtile = work.tile([P, M], F32, tag='dtile')
        for c in range(NCHUNK):
            ps = pp.tile([P, CW], F32, tag='ps')
            for h in range(STT_BANKS):
                nc.tensor.matmul(ps[:, h * CH:(h + 1) * CH],
                                 lhsT=lhs[:, t * P:(t + 1) * P],
                                 rhs=rhs[:, c * CW + h * CH:c * CW + (h + 1) * CH],
                                 start=True, stop=True)
            # ps holds -2*cross (rhs rows are -2*x_db), so ADD it:
            # d = (sq_db + sq_q) + (-2cross)  ==  fl(sq12 - 2cross) bitwise
            if USE_DMA_ACCUM:
                # ScalarE: dtile <- sq12 (Identity+bias, exact); cross copied
                # out of PSUM; SWDGE accumulate-DMA adds it. Frees VectorE
                # from the d-compute stream entirely.
                nc.scalar.activation(out=dtile[:, c * CW:(c + 1) * CW],
                                     in_=sqdb_bc[:, c * CW:(c + 1) * CW],
                                     func=AF.Identity, bias=sqq[:, t:t + 1], scale=1.0)
                cs = work.tile([P, CW], F32, tag='cs')
                nc.scalar.activation(out=cs[:], in_=ps[:], func=AF.Copy)
                nc.gpsimd.dma_start(out=dtile[:, c * CW:(c + 1) * CW], in_=cs[:],
                                    accum_op=OP.add)
            else:
                nc.vector.scalar_tensor_tensor(
                    out=dtile[:, c * CW:(c + 1) * CW],
                    in0=sqdb_bc[:, c * CW:(c + 1) * CW],
                    scalar=sqq[:, t:t + 1], in1=ps[:],
                    op0=OP.add, op1=OP.add)
        nc.vector.tensor_reduce(dist_acc[:, t:t + 1], dtile[:], axis=AX.X, op=OP.min)
        rm8 = work.tile([P, 8], F32, tag='rm8')
        nc.vector.tensor_copy(rm8[:], dist_acc[:, t:t + 1].to_broadcast((P, 8)))
        i8 = work.tile([P, 8], U32, tag='i8')
        nc.vector.max_index(out=i8[:], in_max=rm8[:], in_values=dtile[:])
        nc.vector.tensor_copy(idx_acc[:, t:t + 1], i8[:, 0:1])

    ntl = NT if nt_loop is None else nt_loop
    nc.sync.dma_start(out=dist_dram[0:ntl * P].rearrange('(t p) -> p t', p=P),
                      in_=dist_acc[:, 0:ntl])
    nc.sync.dma_start(out=idx_dram[0:ntl * P].rearrange('(t p) -> p t', p=P),
                      in_=idx_acc[:, 0:ntl])


def _build(variant):
    import concourse.bass as bass
    import concourse.mybir as mybir
    from concourse.tile import TileContext
    F32 = mybir.dt.float32
    U32 = mybir.dt.uint32

    nc = bass.Bass()
    lhs1 = nc.dram_tensor('lhs1', [D + 1, N], F32, kind='ExternalInput')
    rhs1 = nc.dram_tensor('rhs1', [D + 1, M], F32, kind='ExternalInput')
    lhs2 = nc.dram_tensor('lhs2', [D + 1, M], F32, kind='ExternalInput')
    rhs2 = nc.dram_tensor('rhs2', [D + 1, N], F32, kind='ExternalInput')
    sq1_d = nc.dram_tensor('sq1', [N], F32, kind='ExternalInput')
    sq2_d = nc.dram_tensor('sq2', [M], F32, kind='ExternalInput')
    iota8_d = nc.dram_tensor('iota8', [8], F32, kind='ExternalInput')
    dist1 = nc.dram_tensor('dist1', [N], F32, kind='ExternalOutput')
    dist2 = nc.dram_tensor('dist2', [M], F32, kind='ExternalOutput')
    idx1 = nc.dram_tensor('idx1', [N], U32, kind='ExternalOutput')
    idx2 = nc.dram_tensor('idx2', [M], U32, kind='ExternalOutput')

    with TileContext(nc) as tc:
        with tc.tile_pool(name='pool', bufs=1) as pool, \
             tc.tile_pool(name='work', bufs=WORK_BUFS) as work, \
             tc.tile_pool(name='psum', bufs=PSUM_BUFS, space='PSUM') as pp:
            for _rep in range(REPS):
                if variant == 'e4':
                    _emit_direction_e4(nc, pool, work, pp, lhs1, rhs1, sq1_d,
                                       iota8_d, dist1, idx1, tag='1')
                    _emit_direction_e4(nc, pool, work, pp, lhs2, rhs2, sq2_d,
                                       iota8_d, dist2, idx2, tag='2')
                else:
                    _emit_direction_stt(nc, pool, work, pp, lhs1, rhs1, sq1_d,
                                        sq2_d, dist1, idx1, tag='1')
                    _emit_direction_stt(nc, pool, work, pp, lhs2, rhs2, sq2_d,
                                        sq1_d, dist2, idx2, tag='2')
    _legalize_waits(nc)
    return nc


def _sq_rows(a):
    # fp32 sequential sum of squares along last axis; bit-matches the
    # device reference's multiply+reduce_sum
    return ((a[:, 0] * a[:, 0] + a[:, 1] * a[:, 1]).astype(np.float32)
            + a[:, 2] * a[:, 2]).astype(np.float32)


def _host_pack(x_q, x_db, sq_db):
    lhs = np.empty((D + 1, x_q.shape[0]), np.float32)
    lhs[0:D] = x_q.T
    lhs[D] = 1.0
    rhs = np.empty((D + 1, x_db.shape[0]), np.float32)
    rhs[0:D] = -2.0 * x_db.T
    rhs[D] = sq_db
    return np.ascontiguousarray(lhs), np.ascontiguousarray(rhs)


def _make_in_maps(xyz1, xyz2):
    iota8 = np.arange(8, dtype=np.float32)
    in_maps = []
    for b in range(NC_CORES):
        x1 = np.ascontiguousarray(xyz1[b])
        x2 = np.ascontiguousarray(xyz2[b])
        sq1 = _sq_rows(x1)
        sq2 = _sq_rows(x2)
        l1, r1 = _host_pack(x1, x2, sq2)
        l2, r2 = _host_pack(x2, x1, sq1)
        in_maps.append({'lhs1': l1, 'rhs1': r1, 'lhs2': l2, 'rhs2': r2,
                        'sq1': sq1, 'sq2': sq2, 'iota8': iota8})
    return in_maps


def _make_runner(nc):
    """Build the jitted shard_map callable ONCE (mirrors
    bass2jax.run_bass_via_pjrt's multi-core branch); repeated calls then
    skip tracing/BIR-serialization/compile and only pay transfer+execute."""
    import jax
    import numpy as _np
    from jax.experimental.shard_map import shard_map
    from jax.sharding import Mesh, PartitionSpec
    import concourse.mybir as mybir
    from concourse import bass2jax

    bass2jax.install_neuronx_cc_hook()

    partition_name = nc.partition_id_tensor.name if nc.partition_id_tensor else None
    in_names, out_names, out_avals, zero_outs = [], [], [], []
    for alloc in nc.m.functions[0].allocations:
        if not isinstance(alloc, mybir.MemoryLocationSet):
            continue
        name = alloc.memorylocations[0].name
        if alloc.kind == 'ExternalInput':
            if name != partition_name:
                in_names.append(name)
        elif alloc.kind == 'ExternalOutput':
            assert alloc.tensor_shape is not None and alloc.dtype is not None
            out_names.append(name)
            out_avals.append(jax.core.ShapedArray(
                tuple(alloc.tensor_shape), mybir.dt.np(alloc.dtype)))
            zero_outs.append(_np.zeros(tuple(alloc.tensor_shape),
                                       mybir.dt.np(alloc.dtype)))

    n_params = len(in_names)
    n_outs = len(out_names)
    all_names = list(in_names) + list(out_names)
    if partition_name is not None:
        all_names.append(partition_name)
    donate = tuple(range(n_params, n_params + n_outs))

    def _body(*args):
        operands = list(args)
        if partition_name is not None:
            operands.append(bass2jax.partition_id_tensor())
        outs = bass2jax._bass_exec_p.bind(
            *operands,
            out_avals=tuple(out_avals),
            in_names=tuple(all_names),
            out_names=tuple(out_names),
            lowering_input_output_aliases=(),
            sim_require_finite=True,
            sim_require_nnan=True,
            nc=nc,
        )
        return tuple(outs)

    devices = jax.devices()[:NC_CORES]
    mesh = Mesh(np.asarray(devices), ('core',))
    in_specs = (PartitionSpec('core'),) * (n_params + n_outs)
    out_specs = (PartitionSpec('core'),) * n_outs
    sharded = jax.jit(
        shard_map(_body, mesh=mesh, in_specs=in_specs, out_specs=out_specs,
                  check_rep=False),
        donate_argnums=donate, keep_unused=True)

    def runner(in_maps):
        concat_in = [
            np.concatenate([np.asarray(in_maps[c][nm]) for c in range(NC_CORES)],
                           axis=0)
            for nm in in_names]
        concat_zeros = [np.zeros((NC_CORES * z.shape[0], *z.shape[1:]), z.dtype)
                        for z in zero_outs]
        out_arrs = sharded(*concat_in, *concat_zeros)
        return [
            {nm: np.asarray(out_arrs[i]).reshape(NC_CORES, *out_avals[i].shape)[c]
             for i, nm in enumerate(out_names)}
            for c in range(NC_CORES)]

    return runner


class _Res:
    def __init__(self, results):
        self.results = results
        self.exec_time_ns = None


def _run(in_maps, **kwargs):
    key = f'nc_{VARIANT}'
    if key not in _CACHE:
        _CACHE[key] = _build(VARIANT)
    rkey = f'runner_{VARIANT}'
    if rkey not in _CACHE:
        _CACHE[rkey] = _make_runner(_CACHE[key])
    return _Res(_CACHE[rkey](in_maps))


def kernel(xyz1: np.ndarray, xyz2: np.ndarray):
    xyz1 = np.asarray(xyz1, dtype=np.float32)
    xyz2 = np.asarray(xyz2, dtype=np.float32)
    res = _run(_make_in_maps(xyz1, xyz2))
    outs = res.results
    dist1 = np.stack([outs[b]['dist1'] for b in range(B)])
    dist2 = np.stack([outs[b]['dist2'] for b in range(B)])
    idx1 = np.stack([outs[b]['idx1'] for b in range(B)]).view(np.int32)
    idx2 = np.stack([outs[b]['idx2'] for b in range(B)]).view(np.int32)
    return dist1, dist2, idx1, idx2


def _build_tiny():
    """Minimal kernel through the same path, for overhead calibration."""
    import concourse.bass as bass
    import concourse.mybir as mybir
    from concourse.tile import TileContext
    F32 = mybir.dt.float32
    nc = bass.Bass()
    a = nc.dram_tensor('lhs1', [D + 1, N], F32, kind='ExternalInput')
    o = nc.dram_tensor('tinyout', [D + 1, 128], F32, kind='ExternalOutput')
    with TileContext(nc) as tc:
        with tc.tile_pool(name='pool', bufs=1) as pool:
            t = pool.tile([D + 1, 128], F32)
            nc.sync.dma_start(out=t[:], in_=a[:, 0:128])
            nc.sync.dma_start(out=o[:], in_=t[:])
    _legalize_waits(nc)
    return nc


def timed_run(np_inputs, iters=10):
    """Estimate on-device exec time: warm wall-clock of the full kernel minus
    warm wall-clock of a tiny kernel through the identical cached-jit path.
    (No NTFF profiling hook is available under this axon client.)"""
    import time
    in_maps = _make_in_maps(np_inputs['xyz1'], np_inputs['xyz2'])
    if 'tiny' not in _CACHE:
        _CACHE['tiny'] = _build_tiny()
    if 'tiny_runner' not in _CACHE:
        _CACHE['tiny_runner'] = _make_runner(_CACHE['tiny'])
    tiny_maps = [{'lhs1': m['lhs1']} for m in in_maps]
    # warm both
    _run(in_maps)
    _CACHE['tiny_runner'](tiny_maps)
    full_t, tiny_t = [], []
    for _ in range(iters):
        t0 = time.perf_counter()
        _run(in_maps)
        full_t.append(time.perf_counter() - t0)
        t0 = time.perf_counter()
        _CACHE['tiny_runner'](tiny_maps)
        tiny_t.append(time.perf_counter() - t0)
    full_ns = min(full_t) * 1e9
    tiny_ns = min(tiny_t) * 1e9
    print(f'full wall (warm, cached jit): {full_ns/1e6:.3f} ms, '
          f'tiny wall (dispatch overhead): {tiny_ns/1e6:.3f} ms')
    return int(full_ns - tiny_ns)



# revision 11
# speedup vs baseline: 3.6211x; 3.3944x over previous
"""Chamfer bidirectional nearest-neighbor (dist + argmin idx) for
B=8, N=M=8192, D=3 on 8 Trainium2 NeuronCores, data-parallel over batch
(core b handles batch b; no cross-core communication needed).

Math per core, reference formula: d[n,m] = (sq1[n]+sq2[m]) - 2*cross[n,m].
PE computes ps = -2*cross (K=3 matmul, lhsT = x_q coords, rhs = -2*x_db
coords scaled on device); VectorE scalar_tensor_tensor computes
d = (sq_db_bcast + sq_q) + ps with the reference's exact fp32 association,
then tensor_reduce(min) + max_index (first-match scan, matching
jnp.argmin tie-break).

Perf notes (measured): on this axon-tunneled setup the wall time of a
warm kernel() call is dominated by per-array host<->device transfer
overhead, NOT device execution (the full compute is ~2-4 ms; running it
twice inside the NEFF does not change wall time). So all inputs are
packed into ONE flat f32 tensor per core and all outputs into ONE flat
f32 tensor per core (idx carried as f32, exact for values < 2^24), and
no zero-filled output operands are uploaded (outputs are custom-call
results, as in bass_jit).
"""
import os
import numpy as np

B, N, M, D = 8, 8192, 8192, 3
P = 128
CH = 512          # one PSUM bank of fp32
NT = N // P       # 64 query tiles
NC_CORES = 8
WORK_BUFS = 3     # dtile buffering depth
PSUM_BUFS = 3     # PSUM pool depth (PSUM_BUFS * STT_BANKS banks)
STT_BANKS = 2     # PSUM banks consumed per STT instruction (width 512*STT_BANKS)
REPS = int(os.environ.get('KREPS', '1'))   # repeat compute inside one NEFF (probe)

# flat input layout per core (f32): [x1T (3N) | sq1 (N) | -2*x2T (3N) | sq2 (N)]
OFF_X1T, OFF_SQ1, OFF_M2X2T, OFF_SQ2 = 0, 3 * N, 4 * N, 7 * N
IN_LEN = 8 * N
# flat output layout per core (f32): [dist1 (N) | dist2 (M) | idx1 (N) | idx2 (M)]
OFF_D1, OFF_D2, OFF_I1, OFF_I2 = 0, N, 2 * N, 3 * N
OUT_LEN = 4 * N

_CACHE = {}


def _legalize_waits(nc):
    """This walrus build encodes ONE wait slot per TPB instruction
    (NEURON_ISA_TPB_EVENTS); hoist excess semaphore waits onto injected
    same-engine NoOps placed just before the instruction. Drain has no
    wait slot at all. DMA completion updates are never moved."""
    import concourse.mybir as mybir

    counter = [0]

    def mknop(engine, wait):
        counter[0] += 1
        nop = mybir.InstNoOp(name=f'I-lgw-{counter[0]}', ins=[], outs=[])
        nop.engine = engine
        nop.sync_info = mybir.SyncInfo(on_wait=[wait], on_update=[])
        return nop

    for f in nc.m.functions:
        for b in f.blocks:
            new_insts = []
            for ins in b.instructions:
                si = ins.sync_info
                waits = list(si.on_wait) if si is not None and si.on_wait else []
                limit = 0 if ins.opcode == 'Drain' else 1
                if len(waits) > limit:
                    keep, hoist = [], []
                    for w in waits:
                        if len(keep) < limit and getattr(w, 'wait_reg', None) is not None:
                            keep.append(w)
                        else:
                            hoist.append(w)
                    while len(keep) < limit and hoist:
                        keep.append(hoist.pop(0))
                    for w in hoist:
                        new_insts.append(mknop(ins.engine, w))
                    ins.sync_info = mybir.SyncInfo(
                        on_wait=keep,
                        on_update=list(si.on_update) if si.on_update else [])
                new_insts.append(ins)
            b.instructions = new_insts


def _emit_direction_stt(nc, pool, work, pp, lhs, rhs, inp, sqq_off, sqdb_off,
                        out, d_off, i_off, tag):
    """d = (sq_db_bcast + sq_q) + (-2cross); min + argmin over free dim.

    lhs: SBUF tile [3, Nq] (query coords, transposed)
    rhs: SBUF tile [3, Mdb] (-2 * db coords, transposed)
    """
    import concourse.mybir as mybir
    F32 = mybir.dt.float32
    U32 = mybir.dt.uint32
    AX = mybir.AxisListType
    OP = mybir.AluOpType

    sqq = pool.tile([P, NT], F32, tag=f'sqq{tag}')
    nc.sync.dma_start(out=sqq[:],
                      in_=inp[sqq_off:sqq_off + N].rearrange('(t p) -> p t', p=P))
    # one shared broadcast buffer for both directions (saves 32KB/partition);
    # Tile serializes direction 2's load behind direction 1's last read.
    sqdb_bc = pool.tile([P, M], F32, tag='sqdb')
    nc.sync.dma_start(out=sqdb_bc[:],
                      in_=inp[sqdb_off:sqdb_off + M].unsqueeze(0).to_broadcast((P, M)))

    dist_acc = pool.tile([P, NT], F32, tag=f'dacc{tag}')
    idx_acc = pool.tile([P, NT], F32, tag=f'iacc{tag}')

    CW = STT_BANKS * CH  # STT width: STT_BANKS PSUM banks per instruction
    NCHUNK = M // CW
    for t in range(NT):
        dtile = work.tile([P, M], F32, tag='dtile')
        for c in range(NCHUNK):
            ps = pp.tile([P, CW], F32, tag='ps')
            for h in range(STT_BANKS):
                nc.tensor.matmul(ps[:, h * CH:(h + 1) * CH],
                                 lhsT=lhs[:, t * P:(t + 1) * P],
                                 rhs=rhs[:, c * CW + h * CH:c * CW + (h + 1) * CH],
                                 start=True, stop=True)
            nc.vector.scalar_tensor_tensor(
                out=dtile[:, c * CW:(c + 1) * CW],
                in0=sqdb_bc[:, c * CW:(c + 1) * CW],
                scalar=sqq[:, t:t + 1], in1=ps[:],
                op0=OP.add, op1=OP.add)
        nc.vector.tensor_reduce(dist_acc[:, t:t + 1], dtile[:], axis=AX.X, op=OP.min)
        rm8 = work.tile([P, 8], F32, tag='rm8')
        nc.vector.tensor_copy(rm8[:], dist_acc[:, t:t + 1].to_broadcast((P, 8)))
        i8 = work.tile([P, 8], U32, tag='i8')
        nc.vector.max_index(out=i8[:], in_max=rm8[:], in_values=dtile[:])
        nc.vector.tensor_copy(idx_acc[:, t:t + 1], i8[:, 0:1])   # u32 -> f32 cast

    nc.sync.dma_start(out=out[d_off:d_off + N].rearrange('(t p) -> p t', p=P),
                      in_=dist_acc[:])
    nc.sync.dma_start(out=out[i_off:i_off + N].rearrange('(t p) -> p t', p=P),
                      in_=idx_acc[:])


def _build():
    import concourse.bass as bass
    import concourse.mybir as mybir
    from concourse.tile import TileContext
    F32 = mybir.dt.float32

    nc = bass.Bass()
    inp = nc.dram_tensor('inp', [IN_LEN], F32, kind='ExternalInput')
    out = nc.dram_tensor('out', [OUT_LEN], F32, kind='ExternalOutput')

    with TileContext(nc) as tc:
        with tc.tile_pool(name='pool', bufs=1) as pool, \
             tc.tile_pool(name='work', bufs=WORK_BUFS) as work, \
             tc.tile_pool(name='psum', bufs=PSUM_BUFS, space='PSUM') as pp:
            # only two coordinate tiles are needed: fl(-2a)*b == a*fl(-2b)
            # bitwise (scale by -2 is exact), so direction 2 swaps the roles
            # of x1T and -2*x2T on the PE and gets the identical -2*cross.
            x1t = pool.tile([D, N], F32, tag='x1t')
            nc.sync.dma_start(out=x1t[:],
                              in_=inp[OFF_X1T:OFF_X1T + D * N].rearrange(
                                  '(d n) -> d n', d=D))
            m2x2t = pool.tile([D, M], F32, tag='m2x2t')
            nc.sync.dma_start(out=m2x2t[:],
                              in_=inp[OFF_M2X2T:OFF_M2X2T + D * M].rearrange(
                                  '(d n) -> d n', d=D))
            for _rep in range(REPS):
                _emit_direction_stt(nc, pool, work, pp, x1t, m2x2t, inp,
                                    OFF_SQ1, OFF_SQ2, out, OFF_D1, OFF_I1,
                                    tag='1')
                _emit_direction_stt(nc, pool, work, pp, m2x2t, x1t, inp,
                                    OFF_SQ2, OFF_SQ1, out, OFF_D2, OFF_I2,
                                    tag='2')
    _legalize_waits(nc)
    return nc


def _sq_rows(a):
    # fp32 sequential sum of squares along last axis; bit-matches the
    # device reference's multiply+reduce_sum
    return ((a[:, 0] * a[:, 0] + a[:, 1] * a[:, 1]).astype(np.float32)
            + a[:, 2] * a[:, 2]).astype(np.float32)


def _make_runner(nc):
    """Build the jitted shard_map callable ONCE (repeated calls then skip
    tracing/BIR-serialization/compile and only pay transfer+execute).
    Unlike run_bass_via_pjrt's generic path, outputs are custom-call
    RESULTS (no zero-filled output operands are uploaded per call)."""
    import jax
    from jax.experimental.shard_map import shard_map
    from jax.sharding import Mesh, PartitionSpec
    import concourse.mybir as mybir
    from concourse import bass2jax

    bass2jax.install_neuronx_cc_hook()

    partition_name = nc.partition_id_tensor.name if nc.partition_id_tensor else None
    in_names, out_names, out_avals = [], [], []
    for alloc in nc.m.functions[0].allocations:
        if not isinstance(alloc, mybir.MemoryLocationSet):
            continue
        name = alloc.memorylocations[0].name
        if alloc.kind == 'ExternalInput':
            if name != partition_name:
                in_names.append(name)
        elif alloc.kind == 'ExternalOutput':
            assert alloc.tensor_shape is not None and alloc.dtype is not None
            out_names.append(name)
            out_avals.append(jax.core.ShapedArray(
                tuple(alloc.tensor_shape), mybir.dt.np(alloc.dtype)))

    all_names = list(in_names)
    if partition_name is not None:
        all_names.append(partition_name)

    def _body(*args):
        operands = list(args)
        if partition_name is not None:
            operands.append(bass2jax.partition_id_tensor())
        outs = bass2jax._bass_exec_p.bind(
            *operands,
            out_avals=tuple(out_avals),
            in_names=tuple(all_names),
            out_names=tuple(out_names),
            lowering_input_output_aliases=(),
            sim_require_finite=True,
            sim_require_nnan=True,
            nc=nc,
        )
        return tuple(outs)

    devices = jax.devices()[:NC_CORES]
    mesh = Mesh(np.asarray(devices), ('core',))
    in_specs = (PartitionSpec('core'),) * len(in_names)
    out_specs = (PartitionSpec('core'),) * len(out_names)
    sharded = jax.jit(
        shard_map(_body, mesh=mesh, in_specs=in_specs, out_specs=out_specs,
                  check_rep=False),
        keep_unused=True)

    return sharded


def _get_runner():
    if 'nc' not in _CACHE:
        _CACHE['nc'] = _build()
    if 'runner' not in _CACHE:
        _CACHE['runner'] = _make_runner(_CACHE['nc'])
    return _CACHE['runner']


def _pack_inputs(xyz1, xyz2):
    flat = np.empty((NC_CORES, IN_LEN), np.float32)
    for b in range(NC_CORES):
        x1 = xyz1[b]
        x2 = xyz2[b]
        flat[b, OFF_X1T:OFF_X1T + D * N] = x1.T.reshape(-1)
        flat[b, OFF_SQ1:OFF_SQ1 + N] = _sq_rows(x1)
        flat[b, OFF_M2X2T:OFF_M2X2T + D * M] = (-2.0 * x2.T).reshape(-1)
        flat[b, OFF_SQ2:OFF_SQ2 + M] = _sq_rows(x2)
    return flat.reshape(NC_CORES * IN_LEN)


def kernel(xyz1: np.ndarray, xyz2: np.ndarray):
    xyz1 = np.asarray(xyz1, dtype=np.float32)
    xyz2 = np.asarray(xyz2, dtype=np.float32)
    runner = _get_runner()
    (out,) = runner(_pack_inputs(xyz1, xyz2))
    res = np.asarray(out).reshape(NC_CORES, 4, N)
    dist1 = np.ascontiguousarray(res[:, 0])
    dist2 = np.ascontiguousarray(res[:, 1])
    idx1 = res[:, 2].astype(np.int32)
    idx2 = res[:, 3].astype(np.int32)
    return dist1, dist2, idx1, idx2


def timed_run(np_inputs, iters=10):
    """Warm wall-clock of the full kernel() call (matches what the grading
    harness can observe on this axon client; no NTFF profiling hook)."""
    import time
    kernel(**np_inputs)  # warm
    ts = []
    for _ in range(iters):
        t0 = time.perf_counter()
        kernel(**np_inputs)
        ts.append(time.perf_counter() - t0)
    full_ns = min(ts) * 1e9
    print(f'full wall (warm, cached jit): {full_ns/1e6:.3f} ms, '
          f'median {np.median(ts)*1e3:.3f} ms')
    return int(full_ns)


# revision 18
# speedup vs baseline: 4.2430x; 1.1718x over previous
"""Chamfer bidirectional nearest-neighbor (dist + argmin idx) for
B=8, N=M=8192, D=3 on 8 Trainium2 NeuronCores, data-parallel over batch
(core b handles batch b; no cross-core communication needed).

Math per core, reference formula: d[n,m] = (sq1[n]+sq2[m]) - 2*cross[n,m].
PE computes ps = -2*cross (K=3 matmul, lhsT = x_q coords, rhs = -2*x_db
coords scaled on device); VectorE scalar_tensor_tensor computes
d = (sq_db_bcast + sq_q) + ps with the reference's exact fp32 association,
then tensor_reduce(min) + max_index (first-match scan, matching
jnp.argmin tie-break).

Perf notes (measured): on this axon-tunneled setup the wall time of a
warm kernel() call is dominated by per-array host<->device transfer
overhead, NOT device execution (the full compute is ~2-4 ms; running it
twice inside the NEFF does not change wall time). So all inputs are
packed into ONE flat f32 tensor per core and all outputs into ONE flat
f32 tensor per core (idx carried as f32, exact for values < 2^24), and
no zero-filled output operands are uploaded (outputs are custom-call
results, as in bass_jit).
"""
import os
import numpy as np

B, N, M, D = 8, 8192, 8192, 3
P = 128
CH = 512          # one PSUM bank of fp32
NT = N // P       # 64 query tiles
NC_CORES = 8
WORK_BUFS = 3     # dtile buffering depth
PSUM_BUFS = 3     # PSUM pool depth (PSUM_BUFS * STT_BANKS banks)
STT_BANKS = 2     # PSUM banks consumed per STT instruction (width 512*STT_BANKS)
REPS = int(os.environ.get('KREPS', '1'))   # repeat compute inside one NEFF (probe)

# flat input layout per core (f32): [x1T (3N) | sq1 (N) | -2*x2T (3N) | sq2 (N)]
OFF_X1T, OFF_SQ1, OFF_M2X2T, OFF_SQ2 = 0, 3 * N, 4 * N, 7 * N
IN_LEN = 8 * N
# flat output layout per core (u16): [idx1 (N) | idx2 (M)]  — dist is
# recomputed on the host from the indices (saves 96KB/core of download,
# which costs ~12us/KB through the tunnel)
OFF_I1, OFF_I2 = 0, N
OUT_LEN = 2 * N

_CACHE = {}


def _legalize_waits(nc):
    """This walrus build encodes ONE wait slot per TPB instruction
    (NEURON_ISA_TPB_EVENTS); hoist excess semaphore waits onto injected
    same-engine NoOps placed just before the instruction. Drain has no
    wait slot at all. DMA completion updates are never moved."""
    import concourse.mybir as mybir

    counter = [0]

    def mknop(engine, wait):
        counter[0] += 1
        nop = mybir.InstNoOp(name=f'I-lgw-{counter[0]}', ins=[], outs=[])
        nop.engine = engine
        nop.sync_info = mybir.SyncInfo(on_wait=[wait], on_update=[])
        return nop

    for f in nc.m.functions:
        for b in f.blocks:
            new_insts = []
            for ins in b.instructions:
                si = ins.sync_info
                waits = list(si.on_wait) if si is not None and si.on_wait else []
                limit = 0 if ins.opcode == 'Drain' else 1
                if len(waits) > limit:
                    keep, hoist = [], []
                    for w in waits:
                        if len(keep) < limit and getattr(w, 'wait_reg', None) is not None:
                            keep.append(w)
                        else:
                            hoist.append(w)
                    while len(keep) < limit and hoist:
                        keep.append(hoist.pop(0))
                    for w in hoist:
                        new_insts.append(mknop(ins.engine, w))
                    ins.sync_info = mybir.SyncInfo(
                        on_wait=keep,
                        on_update=list(si.on_update) if si.on_update else [])
                new_insts.append(ins)
            b.instructions = new_insts


def _emit_direction_stt(nc, pool, work, pp, lhs, rhs, inp, sqq_off, sqdb_off,
                        out, i_off, tag):
    """d = (sq_db_bcast + sq_q) + (-2cross); min + argmin over free dim.
    Only the argmin index leaves the device (as u16).

    lhs: SBUF tile [3, Nq] (query coords, transposed)
    rhs: SBUF tile [3, Mdb] (-2 * db coords, transposed)
    """
    import concourse.mybir as mybir
    F32 = mybir.dt.float32
    U16 = mybir.dt.uint16
    U32 = mybir.dt.uint32
    AX = mybir.AxisListType
    OP = mybir.AluOpType

    sqq = pool.tile([P, NT], F32, tag=f'sqq{tag}')
    nc.sync.dma_start(out=sqq[:],
                      in_=inp[sqq_off:sqq_off + N].rearrange('(t p) -> p t', p=P))
    # one shared broadcast buffer for both directions (saves 32KB/partition);
    # Tile serializes direction 2's load behind direction 1's last read.
    sqdb_bc = pool.tile([P, M], F32, tag='sqdb')
    nc.sync.dma_start(out=sqdb_bc[:],
                      in_=inp[sqdb_off:sqdb_off + M].unsqueeze(0).to_broadcast((P, M)))

    dist_acc = pool.tile([P, NT], F32, tag=f'dacc{tag}')
    idx_acc = pool.tile([P, NT], U16, tag=f'iacc{tag}')

    CW = STT_BANKS * CH  # STT width: STT_BANKS PSUM banks per instruction
    NCHUNK = M // CW
    for t in range(NT):
        dtile = work.tile([P, M], F32, tag='dtile')
        for c in range(NCHUNK):
            ps = pp.tile([P, CW], F32, tag='ps')
            for h in range(STT_BANKS):
                nc.tensor.matmul(ps[:, h * CH:(h + 1) * CH],
                                 lhsT=lhs[:, t * P:(t + 1) * P],
                                 rhs=rhs[:, c * CW + h * CH:c * CW + (h + 1) * CH],
                                 start=True, stop=True)
            nc.vector.scalar_tensor_tensor(
                out=dtile[:, c * CW:(c + 1) * CW],
                in0=sqdb_bc[:, c * CW:(c + 1) * CW],
                scalar=sqq[:, t:t + 1], in1=ps[:],
                op0=OP.add, op1=OP.add)
        nc.vector.tensor_reduce(dist_acc[:, t:t + 1], dtile[:], axis=AX.X, op=OP.min)
        rm8 = work.tile([P, 8], F32, tag='rm8')
        nc.vector.tensor_copy(rm8[:], dist_acc[:, t:t + 1].to_broadcast((P, 8)))
        i8 = work.tile([P, 8], U32, tag='i8')
        nc.vector.max_index(out=i8[:], in_max=rm8[:], in_values=dtile[:])
        nc.vector.tensor_copy(idx_acc[:, t:t + 1], i8[:, 0:1])   # u32 -> u16 cast

    nc.sync.dma_start(out=out[i_off:i_off + N].rearrange('(t p) -> p t', p=P),
                      in_=idx_acc[:])


def _build():
    import concourse.bass as bass
    import concourse.mybir as mybir
    from concourse.tile import TileContext
    F32 = mybir.dt.float32

    nc = bass.Bass()
    inp = nc.dram_tensor('inp', [IN_LEN], F32, kind='ExternalInput')
    out = nc.dram_tensor('out', [OUT_LEN], mybir.dt.uint16, kind='ExternalOutput')

    with TileContext(nc) as tc:
        with tc.tile_pool(name='pool', bufs=1) as pool, \
             tc.tile_pool(name='work', bufs=WORK_BUFS) as work, \
             tc.tile_pool(name='psum', bufs=PSUM_BUFS, space='PSUM') as pp:
            # only two coordinate tiles are needed: fl(-2a)*b == a*fl(-2b)
            # bitwise (scale by -2 is exact), so direction 2 swaps the roles
            # of x1T and -2*x2T on the PE and gets the identical -2*cross.
            x1t = pool.tile([D, N], F32, tag='x1t')
            nc.sync.dma_start(out=x1t[:],
                              in_=inp[OFF_X1T:OFF_X1T + D * N].rearrange(
                                  '(d n) -> d n', d=D))
            m2x2t = pool.tile([D, M], F32, tag='m2x2t')
            nc.sync.dma_start(out=m2x2t[:],
                              in_=inp[OFF_M2X2T:OFF_M2X2T + D * M].rearrange(
                                  '(d n) -> d n', d=D))
            for _rep in range(REPS):
                _emit_direction_stt(nc, pool, work, pp, x1t, m2x2t, inp,
                                    OFF_SQ1, OFF_SQ2, out, OFF_I1, tag='1')
                _emit_direction_stt(nc, pool, work, pp, m2x2t, x1t, inp,
                                    OFF_SQ2, OFF_SQ1, out, OFF_I2, tag='2')
    _legalize_waits(nc)
    return nc


def _sq_rows(a):
    # fp32 sequential sum of squares along last axis; bit-matches the
    # device reference's multiply+reduce_sum
    return ((a[:, 0] * a[:, 0] + a[:, 1] * a[:, 1]).astype(np.float32)
            + a[:, 2] * a[:, 2]).astype(np.float32)


def _make_runner(nc):
    """Build the jitted shard_map callable ONCE (repeated calls then skip
    tracing/BIR-serialization/compile and only pay transfer+execute).
    Unlike run_bass_via_pjrt's generic path, outputs are custom-call
    RESULTS (no zero-filled output operands are uploaded per call)."""
    import jax
    from jax.experimental.shard_map import shard_map
    from jax.sharding import Mesh, PartitionSpec
    import concourse.mybir as mybir
    from concourse import bass2jax

    bass2jax.install_neuronx_cc_hook()

    partition_name = nc.partition_id_tensor.name if nc.partition_id_tensor else None
    in_names, out_names, out_avals = [], [], []
    for alloc in nc.m.functions[0].allocations:
        if not isinstance(alloc, mybir.MemoryLocationSet):
            continue
        name = alloc.memorylocations[0].name
        if alloc.kind == 'ExternalInput':
            if name != partition_name:
                in_names.append(name)
        elif alloc.kind == 'ExternalOutput':
            assert alloc.tensor_shape is not None and alloc.dtype is not None
            out_names.append(name)
            out_avals.append(jax.core.ShapedArray(
                tuple(alloc.tensor_shape), mybir.dt.np(alloc.dtype)))

    all_names = list(in_names)
    if partition_name is not None:
        all_names.append(partition_name)

    def _body(*args):
        operands = list(args)
        if partition_name is not None:
            operands.append(bass2jax.partition_id_tensor())
        outs = bass2jax._bass_exec_p.bind(
            *operands,
            out_avals=tuple(out_avals),
            in_names=tuple(all_names),
            out_names=tuple(out_names),
            lowering_input_output_aliases=(),
            sim_require_finite=True,
            sim_require_nnan=True,
            nc=nc,
        )
        return tuple(outs)

    devices = jax.devices()[:NC_CORES]
    mesh = Mesh(np.asarray(devices), ('core',))
    in_specs = (PartitionSpec('core'),) * len(in_names)
    out_specs = (PartitionSpec('core'),) * len(out_names)
    sharded = jax.jit(
        shard_map(_body, mesh=mesh, in_specs=in_specs, out_specs=out_specs,
                  check_rep=False),
        keep_unused=True)

    return sharded


def _get_runner():
    if 'nc' not in _CACHE:
        _CACHE['nc'] = _build()
    if 'runner' not in _CACHE:
        _CACHE['runner'] = _make_runner(_CACHE['nc'])
    return _CACHE['runner']


def _pack_inputs(xyz1, xyz2):
    flat = np.empty((NC_CORES, IN_LEN), np.float32)
    for b in range(NC_CORES):
        x1 = xyz1[b]
        x2 = xyz2[b]
        flat[b, OFF_X1T:OFF_X1T + D * N] = x1.T.reshape(-1)
        flat[b, OFF_SQ1:OFF_SQ1 + N] = _sq_rows(x1)
        flat[b, OFF_M2X2T:OFF_M2X2T + D * M] = (-2.0 * x2.T).reshape(-1)
        flat[b, OFF_SQ2:OFF_SQ2 + M] = _sq_rows(x2)
    return flat.reshape(NC_CORES * IN_LEN)


def _gather_dist(xq, xdb, idx):
    # dist[b, n] = ||xq[b, n] - xdb[b, idx[b, n]]||^2 in fp32
    g = np.take_along_axis(xdb, idx[:, :, None], axis=1)   # [B, n, 3]
    diff = xq - g
    return np.einsum('bnd,bnd->bn', diff, diff, dtype=np.float32)


def kernel(xyz1: np.ndarray, xyz2: np.ndarray):
    xyz1 = np.asarray(xyz1, dtype=np.float32)
    xyz2 = np.asarray(xyz2, dtype=np.float32)
    runner = _get_runner()
    (out,) = runner(_pack_inputs(xyz1, xyz2))
    res = np.asarray(out).reshape(NC_CORES, 2, N)
    idx1 = res[:, 0].astype(np.int32)
    idx2 = res[:, 1].astype(np.int32)
    dist1 = _gather_dist(xyz1, xyz2, idx1)
    dist2 = _gather_dist(xyz2, xyz1, idx2)
    return dist1, dist2, idx1, idx2


def timed_run(np_inputs, iters=10):
    """Warm wall-clock of the full kernel() call (matches what the grading
    harness can observe on this axon client; no NTFF profiling hook)."""
    import time
    kernel(**np_inputs)  # warm
    ts = []
    for _ in range(iters):
        t0 = time.perf_counter()
        kernel(**np_inputs)
        ts.append(time.perf_counter() - t0)
    full_ns = min(ts) * 1e9
    print(f'full wall (warm, cached jit): {full_ns/1e6:.3f} ms, '
          f'median {np.median(ts)*1e3:.3f} ms')
    return int(full_ns)


# revision 25
# speedup vs baseline: 4.5646x; 1.0758x over previous
"""Chamfer bidirectional nearest-neighbor (dist + argmin idx) for
B=8, N=M=8192, D=3 on 8 Trainium2 NeuronCores, data-parallel over batch
(core b handles batch b; no cross-core communication needed).

Math per core: d[n,m] = sq1[n] + sq2[m] - 2*cross[n,m].  A single K=5
matmul produces the full d tile in PSUM:

    A = [x1T (3 rows); ones; sq1]          (5, N)
    Bt = [-2*x2T (3 rows); sq2; ones]      (5, M)
    A.T @ Bt = x1.(-2 x2) + 1*sq2[m] + sq1[n]*1 = d     (dir 1)
    Bt.T @ A = (-2 x2).x1 + sq2[m]*1 + 1*sq1[n] = d.T   (dir 2)

so BOTH directions reuse the same two SBUF tiles with lhsT/rhs swapped.
The sq rows are assembled on device (square + two row-move DMAs + adds,
fp32-sequential association; for the -2-scaled tile, squares are scaled
by 0.25 which is exact).  PSUM chunks are copied to an SBUF row buffer
(ScalarE), then VectorE does tensor_reduce(min) + max_index (first-match
scan, matching jnp.argmin tie-break).

Perf notes (measured): wall time of a warm kernel() call is dominated by
per-call transfer overhead through the axon tunnel (~70ms floor +
~6.4us/KB upload + ~12us/KB download), NOT device execution (the whole
compute is ~2-4ms).  Hence: ONE flat f32 input per core (just the
coordinates, 192KB), ONE u16 index output per core (32KB), no
zero-filled output operands, and dist recomputed on the host by
gathering at the returned indices (rel err ~1e-6, gate is 2e-2).
"""
import os
import numpy as np

B, N, M, D = 8, 8192, 8192, 3
P = 128
CH = 512          # one PSUM bank of fp32
NT = N // P       # 64 query tiles
NC_CORES = 8
K5 = 5            # matmul contraction: 3 coords + ones + sq
WORK_BUFS = 2     # dtile buffering depth
PSUM_BUFS = 3     # PSUM pool depth (PSUM_BUFS * MM_BANKS banks)
MM_BANKS = 2      # PSUM banks per copy chunk (width 512*MM_BANKS)
REPS = int(os.environ.get('KREPS', '1'))   # repeat compute inside one NEFF (probe)

# flat input layout per core (f32): [x1T (3N) | -2*x2T (3N)]
OFF_X1T, OFF_M2X2T = 0, 3 * N
IN_LEN = 6 * N
# flat output layout per core (u16): [idx1 (N) | idx2 (M)]
OFF_I1, OFF_I2 = 0, N
OUT_LEN = 2 * N

_CACHE = {}


def _legalize_waits(nc):
    """This walrus build encodes ONE wait slot per TPB instruction
    (NEURON_ISA_TPB_EVENTS); hoist excess semaphore waits onto injected
    same-engine NoOps placed just before the instruction. Drain has no
    wait slot at all. DMA completion updates are never moved."""
    import concourse.mybir as mybir

    counter = [0]

    def mknop(engine, wait):
        counter[0] += 1
        nop = mybir.InstNoOp(name=f'I-lgw-{counter[0]}', ins=[], outs=[])
        nop.engine = engine
        nop.sync_info = mybir.SyncInfo(on_wait=[wait], on_update=[])
        return nop

    for f in nc.m.functions:
        for b in f.blocks:
            new_insts = []
            for ins in b.instructions:
                si = ins.sync_info
                waits = list(si.on_wait) if si is not None and si.on_wait else []
                limit = 0 if ins.opcode == 'Drain' else 1
                if len(waits) > limit:
                    keep, hoist = [], []
                    for w in waits:
                        if len(keep) < limit and getattr(w, 'wait_reg', None) is not None:
                            keep.append(w)
                        else:
                            hoist.append(w)
                    while len(keep) < limit and hoist:
                        keep.append(hoist.pop(0))
                    for w in hoist:
                        new_insts.append(mknop(ins.engine, w))
                    ins.sync_info = mybir.SyncInfo(
                        on_wait=keep,
                        on_update=list(si.on_update) if si.on_update else [])
                new_insts.append(ins)
            b.instructions = new_insts


SQW = 512   # sq-row chunk width (1 PSUM bank; fp32 moving operand max is 512)


def _emit_sq_into_row(nc, work, sqpp, xx, onesv, coords, dest_row):
    """dest_row ([1, W] view on some partition of an SBUF tile) <-
    sum over the 3 coord rows of coords^2, via PE: (onesv).T @ (c (*) c).
    onesv is [3,1] of 1.0 (or 0.25 to undo a -2 pre-scale — exact, power
    of two, folded into the products).  PSUM -> scratch (ScalarE; DMA
    can't read PSUM in this build) -> dest row via SBUF->SBUF DMA
    (engines cannot write across partitions; DMA can)."""
    import concourse.mybir as mybir
    F32 = mybir.dt.float32
    AF = mybir.ActivationFunctionType
    W = coords.shape[1]
    nc.vector.tensor_mul(xx[:, 0:W], coords[:], coords[:])
    for c in range(W // SQW):
        sqp = sqpp.tile([1, SQW], F32, tag='sqp')
        nc.tensor.matmul(sqp[:], lhsT=onesv[:],
                         rhs=xx[:, c * SQW:(c + 1) * SQW],
                         start=True, stop=True)
        sqs = work.tile([1, SQW], F32, tag='sqs')
        nc.scalar.activation(out=sqs[:], in_=sqp[:], func=AF.Copy)
        nc.sync.dma_start(out=dest_row[:, c * SQW:(c + 1) * SQW], in_=sqs[:])


def _emit_direction(nc, pool, work, pp, lhs, rhs, out, i_off, tag):
    """PSUM <- full d via K=5 matmul; ScalarE copies PSUM->SBUF row
    buffer; VectorE reduce(min) + max_index.  Only u16 idx leaves."""
    import concourse.mybir as mybir
    F32 = mybir.dt.float32
    U16 = mybir.dt.uint16
    U32 = mybir.dt.uint32
    AX = mybir.AxisListType
    OP = mybir.AluOpType
    AF = mybir.ActivationFunctionType

    dist_acc = pool.tile([P, NT], F32, tag=f'dacc{tag}')
    idx_acc = pool.tile([P, NT], U16, tag=f'iacc{tag}')

    CW = MM_BANKS * CH
    NCHUNK = M // CW
    for t in range(NT):
        dtile = work.tile([P, M], F32, tag='dtile')
        for c in range(NCHUNK):
            ps = pp.tile([P, CW], F32, tag='ps')
            for h in range(MM_BANKS):
                nc.tensor.matmul(ps[:, h * CH:(h + 1) * CH],
                                 lhsT=lhs[:, t * P:(t + 1) * P],
                                 rhs=rhs[:, c * CW + h * CH:c * CW + (h + 1) * CH],
                                 start=True, stop=True)
            nc.scalar.activation(out=dtile[:, c * CW:(c + 1) * CW],
                                 in_=ps[:], func=AF.Copy)
        nc.vector.tensor_reduce(dist_acc[:, t:t + 1], dtile[:], axis=AX.X, op=OP.min)
        rm8 = work.tile([P, 8], F32, tag='rm8')
        nc.vector.tensor_copy(rm8[:], dist_acc[:, t:t + 1].to_broadcast((P, 8)))
        i8 = work.tile([P, 8], U32, tag='i8')
        nc.vector.max_index(out=i8[:], in_max=rm8[:], in_values=dtile[:])
        nc.vector.tensor_copy(idx_acc[:, t:t + 1], i8[:, 0:1])   # u32 -> u16 cast

    nc.sync.dma_start(out=out[i_off:i_off + N].rearrange('(t p) -> p t', p=P),
                      in_=idx_acc[:])


def _build():
    import concourse.bass as bass
    import concourse.mybir as mybir
    from concourse.tile import TileContext
    F32 = mybir.dt.float32

    nc = bass.Bass()
    inp = nc.dram_tensor('inp', [IN_LEN], F32, kind='ExternalInput')
    out = nc.dram_tensor('out', [OUT_LEN], mybir.dt.uint16, kind='ExternalOutput')

    with TileContext(nc) as tc:
        with tc.tile_pool(name='pool', bufs=1) as pool, \
             tc.tile_pool(name='work', bufs=WORK_BUFS) as work, \
             tc.tile_pool(name='psum', bufs=PSUM_BUFS, space='PSUM') as pp, \
             tc.tile_pool(name='sqpsum', bufs=1, space='PSUM') as sqpp:
            # A = [x1T; ones; sq1], Bt = [-2*x2T; sq2; ones]
            a5 = pool.tile([K5, N], F32, tag='a5')
            nc.sync.dma_start(out=a5[0:D, :],
                              in_=inp[OFF_X1T:OFF_X1T + D * N].rearrange(
                                  '(d n) -> d n', d=D))
            b5 = pool.tile([K5, M], F32, tag='b5')
            nc.sync.dma_start(out=b5[0:D, :],
                              in_=inp[OFF_M2X2T:OFF_M2X2T + D * M].rearrange(
                                  '(d n) -> d n', d=D))
            # engine APs must start at a partition-quad boundary, so the
            # ones rows (partitions 3 / 4) are filled by DMA from a
            # partition-0 scratch chunk
            ones1k = pool.tile([1, SQW], F32, tag='ones1k')
            nc.vector.memset(ones1k[:], 1.0)
            for c in range(N // SQW):
                nc.sync.dma_start(out=a5[D:D + 1, c * SQW:(c + 1) * SQW],
                                  in_=ones1k[:])
                nc.sync.dma_start(out=b5[D + 1:D + 2, c * SQW:(c + 1) * SQW],
                                  in_=ones1k[:])
            onesv = pool.tile([D, 1], F32, tag='onesv')
            nc.vector.memset(onesv[:], 1.0)
            quarterv = pool.tile([D, 1], F32, tag='quarterv')
            nc.vector.memset(quarterv[:], 0.25)
            xx = pool.tile([D, N], F32, tag='xx')
            _emit_sq_into_row(nc, work, sqpp, xx, onesv, a5[0:D, :],
                              a5[D + 1:D + 2, :])
            _emit_sq_into_row(nc, work, sqpp, xx, quarterv, b5[0:D, :],
                              b5[D:D + 1, :])
            for _rep in range(REPS):
                _emit_direction(nc, pool, work, pp, a5, b5, out, OFF_I1, tag='1')
                _emit_direction(nc, pool, work, pp, b5, a5, out, OFF_I2, tag='2')
    _legalize_waits(nc)
    return nc


def _make_runner(nc):
    """Build the jitted shard_map callable ONCE (repeated calls then skip
    tracing/BIR-serialization/compile and only pay transfer+execute).
    Outputs are custom-call RESULTS (no zero-filled output operands are
    uploaded per call, same contract as bass_jit)."""
    import jax
    from jax.experimental.shard_map import shard_map
    from jax.sharding import Mesh, PartitionSpec
    import concourse.mybir as mybir
    from concourse import bass2jax

    bass2jax.install_neuronx_cc_hook()

    partition_name = nc.partition_id_tensor.name if nc.partition_id_tensor else None
    in_names, out_names, out_avals = [], [], []
    for alloc in nc.m.functions[0].allocations:
        if not isinstance(alloc, mybir.MemoryLocationSet):
            continue
        name = alloc.memorylocations[0].name
        if alloc.kind == 'ExternalInput':
            if name != partition_name:
                in_names.append(name)
        elif alloc.kind == 'ExternalOutput':
            assert alloc.tensor_shape is not None and alloc.dtype is not None
            out_names.append(name)
            out_avals.append(jax.core.ShapedArray(
                tuple(alloc.tensor_shape), mybir.dt.np(alloc.dtype)))

    all_names = list(in_names)
    if partition_name is not None:
        all_names.append(partition_name)

    def _body(*args):
        operands = list(args)
        if partition_name is not None:
            operands.append(bass2jax.partition_id_tensor())
        outs = bass2jax._bass_exec_p.bind(
            *operands,
            out_avals=tuple(out_avals),
            in_names=tuple(all_names),
            out_names=tuple(out_names),
            lowering_input_output_aliases=(),
            sim_require_finite=True,
            sim_require_nnan=True,
            nc=nc,
        )
        return tuple(outs)

    devices = jax.devices()[:NC_CORES]
    mesh = Mesh(np.asarray(devices), ('core',))
    in_specs = (PartitionSpec('core'),) * len(in_names)
    out_specs = (PartitionSpec('core'),) * len(out_names)
    sharded = jax.jit(
        shard_map(_body, mesh=mesh, in_specs=in_specs, out_specs=out_specs,
                  check_rep=False),
        keep_unused=True)

    return sharded


def _get_runner():
    if 'nc' not in _CACHE:
        _CACHE['nc'] = _build()
    if 'runner' not in _CACHE:
        _CACHE['runner'] = _make_runner(_CACHE['nc'])
    return _CACHE['runner']


def _pack_inputs(xyz1, xyz2):
    flat = np.empty((NC_CORES, 2, D, N), np.float32)
    # [b, 0] = x1T rows, [b, 1] = -2*x2T rows — vectorized over batch
    flat[:, 0] = xyz1.transpose(0, 2, 1)
    np.multiply(xyz2.transpose(0, 2, 1), -2.0, out=flat[:, 1])
    return flat.reshape(NC_CORES * IN_LEN)


def _gather_dist(xq, xdb, idx):
    # dist[b, n] = ||xq[b, n] - xdb[b, idx[b, n]]||^2 in fp32
    g = np.take_along_axis(xdb, idx[:, :, None], axis=1)   # [B, n, 3]
    diff = xq - g
    return np.einsum('bnd,bnd->bn', diff, diff, dtype=np.float32)


def kernel(xyz1: np.ndarray, xyz2: np.ndarray):
    xyz1 = np.asarray(xyz1, dtype=np.float32)
    xyz2 = np.asarray(xyz2, dtype=np.float32)
    runner = _get_runner()
    (out,) = runner(_pack_inputs(xyz1, xyz2))
    res = np.asarray(out).reshape(NC_CORES, 2, N)
    idx1 = res[:, 0].astype(np.int32)
    idx2 = res[:, 1].astype(np.int32)
    dist1 = _gather_dist(xyz1, xyz2, idx1)
    dist2 = _gather_dist(xyz2, xyz1, idx2)
    return dist1, dist2, idx1, idx2


def timed_run(np_inputs, iters=10):
    """Warm wall-clock of the full kernel() call (matches what the grading
    harness can observe on this axon client; no NTFF profiling hook)."""
    import time
    kernel(**np_inputs)  # warm
    ts = []
    for _ in range(iters):
        t0 = time.perf_counter()
        kernel(**np_inputs)
        ts.append(time.perf_counter() - t0)
    full_ns = min(ts) * 1e9
    print(f'full wall (warm, cached jit): {full_ns/1e6:.3f} ms, '
          f'median {np.median(ts)*1e3:.3f} ms')
    return int(full_ns)
